# revision 1
# baseline (speedup 1.0000x reference)
"""Trainium2 Bass kernel for nn_Attention_66546223284383.

Strategy: pure data-parallel over batch B=16 -> 2 batches per core x 8 cores.
Per core, per batch:
  qkvT = (BN-folded W)^T @ x^T           (h on partitions, n free)
  per head: scoresT = k^T.T @ qT         (m on partitions, n free; attn scale
            folded into q weights), exp on ACT (scores are small: no max-sub),
            oT = [v|1]^T @ expT          (65 rows: 64 o-dims + denominator),
            PE-transpose -> normalize -> stage o to DRAM (n, c) bf16.
  conv branch: v reflowed via DRAM to (channel, spatial), hardswish + 9-tap
            depthwise conv on VectorE with per-partition tap weights.
  proj:     O2^T tiles read back via XBAR transpose DMA (handles the torch
            "raw reshape" (B,N,H,d)->(B,DH,N) as a flat re-chunk),
            xo = O2 @ proj_w^T + vc  (r on partitions, j free)
  out:      out = xo^T @ out_w^T + out_b -> (1024, 512) fp32.
All matmuls bf16 inputs with fp32 PSUM accumulation.
"""
import sys
import numpy as np

sys.path.insert(0, "/opt/trn_rl_repo")

import ml_dtypes  # noqa: E402

BF16 = ml_dtypes.bfloat16

KD, H, D, DH, DIM, IMG, S, N, B = 32, 8, 64, 512, 512, 1024, 32, 1044, 16
EPS = 1e-5
NCORES = 8
BPC = B // NCORES  # batches per core
NT = [(i * 128, 128) for i in range(8)] + [(1024, 20)]   # 1044 partition tiles
NCH = [(0, 512), (512, 512), (1024, 20)]                 # 1044 free chunks

_cached = {}


def _build():
    from concourse import bacc, tile
    import concourse.bass as bass
    import concourse.mybir as mybir
    from concourse.masks import make_identity

    dt = mybir.dt
    alu = mybir.AluOpType
    act_exp = mybir.ActivationFunctionType.Exp

    nc = bacc.Bacc(None, target_bir_lowering=False, debug=False)

    xs = nc.declare_dram_parameter("xs", [BPC, N, DIM], dt.bfloat16, isOutput=False)
    # wqkv cols: 0:768 = 6 padded qk tiles (3 q tiles then 3 k tiles, 3 heads
    # per tile at offsets 0/32/64); 768:1288 = v channels hh*65+j (j==64 is a
    # ones channel: zero weights, bias 1 -> softmax denominator column)
    wqkv = nc.declare_dram_parameter("wqkv", [DIM, 1288], dt.bfloat16, isOutput=False)
    bqkv = nc.declare_dram_parameter("bqkv", [128, 6], dt.float32, isOutput=False)
    bv = nc.declare_dram_parameter("bv", [1, 520], dt.float32, isOutput=False)
    wproj = nc.declare_dram_parameter("wproj", [N, IMG], dt.bfloat16, isOutput=False)
    wout = nc.declare_dram_parameter("wout", [DH, DIM], dt.bfloat16, isOutput=False)
    tapw = nc.declare_dram_parameter("tapw", [128, 36], dt.float32, isOutput=False)
    cbp = nc.declare_dram_parameter("cbp", [128, 4], dt.float32, isOutput=False)
    pbp = nc.declare_dram_parameter("pbp", [1, IMG], dt.float32, isOutput=False)
    obp = nc.declare_dram_parameter("obp", [1, DIM], dt.float32, isOutput=False)
    out_ext = nc.declare_dram_parameter("out", [BPC, IMG, DIM], dt.float32, isOutput=True)

    o_nat = nc.dram_tensor("o_nat", [BPC, N, DH], dt.bfloat16)
    v_dram = nc.dram_tensor("v_dram", [BPC, H, N, D], dt.bfloat16)

    with tile.TileContext(nc) as tc:
        with (
            tc.tile_pool(name="w", bufs=1) as pw,
            tc.tile_pool(name="xT", bufs=8) as pxT,
            tc.tile_pool(name="qkvT", bufs=12) as pqk,
            tc.tile_pool(name="vnat", bufs=18) as pvn,
            tc.tile_pool(name="exp", bufs=3) as pexp,
            tc.tile_pool(name="oTs", bufs=2) as poTs,
            tc.tile_pool(name="small", bufs=4) as psm,
            tc.tile_pool(name="conv", bufs=2) as pcv,
            tc.tile_pool(name="cin", bufs=3) as pcin,
            tc.tile_pool(name="vc", bufs=8) as pvc,
            tc.tile_pool(name="o2t", bufs=1) as po2,
            tc.tile_pool(name="xo", bufs=5) as pxo,
            # PSUM budget (8 banks): sc 2x(128,1024)=4, ot 3x(65,512)=3, mm 1
            tc.tile_pool(name="pssc", bufs=2, space=bass.MemorySpace.PSUM) as pssc,
            tc.tile_pool(name="psmm", bufs=1, space=bass.MemorySpace.PSUM) as psmm,
            tc.tile_pool(name="psot", bufs=3, space=bass.MemorySpace.PSUM) as psot,
        ):
            # ---- constants / weights ----
            id_sb = pw.tile([128, 128], dt.bfloat16, tag="id")
            make_identity(nc, id_sb[:])
            wqkv_sb = pw.tile([128, 4, 1288], dt.bfloat16, tag="wqkv")
            nc.sync.dma_start(wqkv_sb[:], wqkv[:].rearrange("(k p) h -> p k h", p=128))
            bqkv_sb = pw.tile([128, 6], dt.float32, tag="bqkv")
            nc.sync.dma_start(bqkv_sb[:], bqkv[:])
            bv_sb = pw.tile([1, 520], dt.float32, tag="bv")
            nc.sync.dma_start(bv_sb[:], bv[:])
            bvbc = pw.tile([128, 520], dt.float32, tag="bvbc")
            nc.gpsimd.partition_broadcast(bvbc[:], bv_sb[:])
            wproj_sb = []
            for mt, (m0, msz) in enumerate(NT):
                t = pw.tile([msz, 1024], dt.bfloat16, tag=f"wproj{mt}")
                nc.sync.dma_start(t[:], wproj[m0:m0 + msz, :])
                wproj_sb.append(t)
            wout_sb = pw.tile([128, 4, DIM], dt.bfloat16, tag="wout")
            nc.sync.dma_start(wout_sb[:], wout[:].rearrange("(k p) c -> p k c", p=128))
            tapw_sb = pw.tile([128, 36], dt.float32, tag="tapw")
            nc.sync.dma_start(tapw_sb[:], tapw[:])
            cb_sb = pw.tile([128, 4], dt.float32, tag="cb")
            nc.sync.dma_start(cb_sb[:], cbp[:])
            pb_sb = pw.tile([1, IMG], dt.float32, tag="pb")
            nc.sync.dma_start(pb_sb[:], pbp[:])
            ob_sb = pw.tile([1, DIM], dt.float32, tag="ob")
            nc.sync.dma_start(ob_sb[:], obp[:])
            pbbc = pw.tile([128, IMG], dt.float32, tag="pbbc")
            nc.gpsimd.partition_broadcast(pbbc[:], pb_sb[:])
            obbc = pw.tile([128, DIM], dt.float32, tag="obbc")
            nc.gpsimd.partition_broadcast(obbc[:], ob_sb[:])

            qkvT_all, vnat_all, vc_all = {}, {}, {}

            # ======== phase 1: x^T + qk + v for BOTH batches (PE-dense) ========
            for b in range(BPC):
                xT = []
                for cb4 in range(4):
                    t = pxT.tile([128, N], dt.bfloat16, tag="xT")
                    c0 = cb4 * 128
                    nc.sync.dma_start_transpose(t[:, 0:1040], xs[b, 0:1040, c0:c0 + 128])
                    nc.sync.dma_start(
                        t[:, 1040:N], xs[b, 1040:N, c0:c0 + 128].rearrange("a b -> b a")
                    )
                    xT.append(t)

                # 6 padded qk tiles: 0-2 q heads (3/tile @ 0,32,64), 3-5 k heads
                qkvT = []
                for mt6 in range(6):
                    t = pqk.tile([128, N], dt.bfloat16, tag="qkvT")
                    pss = [
                        pssc.tile([128, 512], dt.float32, tag="sc", name=f"qk{c}")
                        for c in range(2)
                    ] + [psmm.tile([128, 20], dt.float32, tag="mm", name="qk2")]
                    for kc in range(4):
                        for c, (ci, cw) in enumerate(NCH):
                            nc.tensor.matmul(
                                pss[c][:, 0:cw] if c < 2 else pss[c][:],
                                wqkv_sb[:, kc, mt6 * 128:(mt6 + 1) * 128],
                                xT[kc][:, ci:ci + cw],
                                start=(kc == 0), stop=(kc == 3),
                            )
                    for c, (ci, cw) in enumerate(NCH):
                        nc.vector.tensor_scalar(
                            t[:, ci:ci + cw],
                            pss[c][:, 0:cw] if c < 2 else pss[c][:],
                            bqkv_sb[:, mt6:mt6 + 1], None, op0=alu.add,
                        )
                    qkvT.append(t)
                qkvT_all[b] = qkvT

                # v directly in (n, 8*65) layout with ones channels
                vnat = []
                for nt, (n0, nsz) in enumerate(NT):
                    t = pvn.tile([nsz, 520], dt.bfloat16, tag="vnat")
                    pss = [
                        pssc.tile([nsz, 512], dt.float32, tag="sc", name=f"vn{c}")
                        for c in range(2)
                    ]
                    for kc in range(4):
                        for c in range(2):
                            nc.tensor.matmul(
                                pss[c][:, 0:260],
                                xT[kc][:, n0:n0 + nsz],
                                wqkv_sb[:, kc, 768 + c * 260:768 + (c + 1) * 260],
                                start=(kc == 0), stop=(kc == 3),
                            )
                    for c in range(2):
                        nc.vector.tensor_tensor(
                            t[:, c * 260:(c + 1) * 260], pss[c][:, 0:260],
                            bvbc[0:nsz, c * 260:(c + 1) * 260], op=alu.add,
                        )
                    # stage all 8 heads' v in ONE DMA on the gpsimd queue
                    # (keeps the serial sync queue short)
                    nc.gpsimd.dma_start(
                        v_dram[b].rearrange("h n d -> n h d")[n0:n0 + nsz],
                        t[:].rearrange("p (h dd) -> p h dd", h=8)[:, :, 0:D],
                    )
                    vnat.append(t)
                vnat_all[b] = vnat

            # ======== phase 2: all 16 heads' attention (+conv on DVE) ========
            for b in range(BPC):
                qkvT, vnat = qkvT_all[b], vnat_all[b]
                vc_tiles = []
                for hh in range(H):
                    # q/k at base partitions {0,32,64} in padded tiles
                    qo = (hh % 3) * KD
                    qT = qkvT[hh // 3][qo:qo + KD, :]
                    kT = qkvT[3 + hh // 3][qo:qo + KD, :]

                    oT_ps = [
                        psot.tile([D + 1, cw], dt.float32, tag="ot", name=f"ot{k}")
                        for k, (_, cw) in enumerate(NCH)
                    ]
                    for mt, (m0, msz) in enumerate(NT):
                        et = pexp.tile([128, N], dt.bfloat16, tag="exp")
                        # scores: one 2-bank psum tile (chunks bank-aligned),
                        # tail rides the mm slot; ONE exp per region
                        sc = pssc.tile([msz, 1024], dt.float32, tag="sc")
                        sct = psmm.tile([msz, 20], dt.float32, tag="mm")
                        for (ci, cw) in ((0, 512), (512, 512)):
                            nc.tensor.matmul(
                                sc[:, ci:ci + cw], kT[:, m0:m0 + msz],
                                qT[:, ci:ci + cw], start=True, stop=True,
                            )
                        nc.tensor.matmul(
                            sct[:], kT[:, m0:m0 + msz], qT[:, 1024:N],
                            start=True, stop=True,
                        )
                        nc.scalar.activation(et[0:msz, 0:1024], sc[:], act_exp)
                        nc.scalar.activation(et[0:msz, 1024:N], sct[:], act_exp)
                        for k, (ci, cw) in enumerate(NCH):
                            nc.tensor.matmul(
                                oT_ps[k][:],
                                vnat[mt][:, hh * 65:hh * 65 + D + 1],
                                et[0:msz, ci:ci + cw],
                                start=(mt == 0), stop=(mt == 8),
                            )
                    oT_sb = poTs.tile([D + 1, N], dt.bfloat16, tag="oTs")
                    for k, (ci, cw) in enumerate(NCH):
                        nc.vector.tensor_copy(oT_sb[:, ci:ci + cw], oT_ps[k][:])
                    # transpose back, normalize, stage o
                    for nt, (n0, nsz) in enumerate(NT):
                        tp2 = psmm.tile([nsz, D + 1], dt.bfloat16, tag="mm")
                        nc.tensor.transpose(
                            tp2[:], oT_sb[:, n0:n0 + nsz], id_sb[0:D + 1, 0:D + 1]
                        )
                        rcp = psm.tile([nsz, 1], dt.float32, tag="rcp")
                        nc.vector.reciprocal(rcp[:], tp2[:, D:D + 1])
                        onrm = psm.tile([nsz, D], dt.bfloat16, tag="onrm")
                        nc.vector.tensor_scalar(
                            onrm[:], tp2[:, 0:D], rcp[:], None, op0=alu.mult
                        )
                        nc.gpsimd.dma_start(
                            o_nat[b, n0:n0 + nsz, hh * D:(hh + 1) * D], onrm[:]
                        )

                    # conv per channel-tile once its two heads are staged
                    # (bf16 DVE ops for the 2x mode)
                    if hh % 2 == 1:
                        ct = hh // 2
                        cin = pcin.tile([128, 1024], dt.bfloat16, tag="cin")
                        for hl in range(2):
                            src = v_dram[b, 2 * ct + hl, 0:1024, :].rearrange(
                                "(c n2) d -> c (n2 d)", c=64
                            )
                            nc.scalar.dma_start(cin[hl * 64:(hl + 1) * 64, :], src)
                        u = pcv.tile([128, 1024], dt.bfloat16, tag="u")
                        nc.vector.tensor_scalar(
                            u[:], cin[:], 1.0 / 6.0, 0.5, op0=alu.mult, op1=alu.add
                        )
                        nc.vector.tensor_scalar(
                            u[:], u[:], 1.0, 0.0, op0=alu.min, op1=alu.max
                        )
                        hs = pcv.tile([128, 32, 32], dt.bfloat16, tag="hs")
                        nc.vector.tensor_tensor(
                            hs[:].rearrange("p a b -> p (a b)"), u[:], cin[:],
                            op=alu.mult,
                        )
                        pad = pcv.tile([128, 34, 34], dt.bfloat16, tag="pad")
                        nc.vector.memset(pad[:], 0.0)
                        nc.vector.tensor_copy(pad[:, 1:33, 1:33], hs[:])
                        acc = pcv.tile([128, 1024], dt.bfloat16, tag="acc")
                        t2 = pcv.tile([128, 1024], dt.bfloat16, tag="t2")
                        for tap in range(9):
                            dy, dx = tap // 3, tap % 3
                            view = pad[:, dy:dy + 32, dx:dx + 32]
                            wsl = tapw_sb[:, ct * 9 + tap:ct * 9 + tap + 1]
                            if tap == 0:
                                nc.vector.tensor_scalar(
                                    acc[:].rearrange("p (a b) -> p a b", a=32),
                                    view, wsl, None, op0=alu.mult,
                                )
                            else:
                                nc.vector.tensor_scalar(
                                    t2[:].rearrange("p (a b) -> p a b", a=32),
                                    view, wsl, None, op0=alu.mult,
                                )
                                nc.vector.tensor_tensor(acc[:], acc[:], t2[:], op=alu.add)
                        nc.vector.tensor_scalar(
                            acc[:], acc[:], cb_sb[:, ct:ct + 1], None, op0=alu.add
                        )
                        vct = pvc.tile([128, 1024], dt.bfloat16, tag="vc")
                        nc.vector.tensor_tensor(vct[:], acc[:], pbbc[:], op=alu.add)
                        vc_tiles.append(vct)
                vc_all[b] = vc_tiles

            # ======== phase 3: proj + out per batch ========
            for b in range(BPC):
                vc_tiles = vc_all[b]
                O2v = o_nat[b].rearrange("n c -> (n c)").rearrange("(r m) -> r m", m=N)
                o2t = []
                for mt, (m0, msz) in enumerate(NT):
                    t = po2.tile([msz, DH], dt.bfloat16, tag=f"o2t{mt}")
                    if msz == 128:
                        nc.sync.dma_start_transpose(t[:], O2v[:, m0:m0 + msz])
                    else:
                        nc.sync.dma_start(
                            t[:], O2v[:, m0:m0 + msz].rearrange("a b -> b a")
                        )
                    o2t.append(t)
                xo_sb = [
                    pxo.tile([128, 1024], dt.bfloat16, tag="xo", name=f"xo{rt}")
                    for rt in range(4)
                ]
                for rt in range(4):
                    pss = [
                        pssc.tile([128, 512], dt.float32, tag="sc", name=f"xop{c}")
                        for c in range(2)
                    ]
                    for mt, (m0, msz) in enumerate(NT):
                        for c, jc in enumerate((0, 512)):
                            nc.tensor.matmul(
                                pss[c][:],
                                o2t[mt][0:msz, rt * 128:(rt + 1) * 128],
                                wproj_sb[mt][0:msz, jc:jc + 512],
                                start=(mt == 0), stop=(mt == 8),
                            )
                    for c, jc in enumerate((0, 512)):
                        nc.vector.tensor_tensor(
                            xo_sb[rt][:, jc:jc + 512], pss[c][:],
                            vc_tiles[rt][:, jc:jc + 512], op=alu.add,
                        )

                for jt in range(8):
                    ps = psmm.tile([128, 512], dt.float32, tag="mm")
                    for rt in range(4):
                        nc.tensor.matmul(
                            ps[:],
                            xo_sb[rt][:, jt * 128:(jt + 1) * 128],
                            wout_sb[:, rt, :],
                            start=(rt == 0), stop=(rt == 3),
                        )
                    osb = psm.tile([128, DIM], dt.float32, tag="outsb")
                    nc.vector.tensor_tensor(osb[:], ps[:], obbc[:], op=alu.add)
                    nc.scalar.dma_start(out_ext[b, jt * 128:(jt + 1) * 128, :], osb[:])

    nc.compile()
    return nc


def _prep_weights(qkv_w, bn_gamma, bn_beta, bn_mean, bn_var,
                  conv_w, conv_b, proj_w, proj_b, out_w, out_b):
    s = bn_gamma / np.sqrt(bn_var + EPS)
    bias = bn_beta - bn_mean * s
    Wt = (qkv_w * s[:, None]).T.copy()
    bvec = bias.copy()
    scale = KD ** -0.5
    for hh in range(H):
        Wt[:, hh * 128:hh * 128 + KD] *= scale
        bvec[hh * 128:hh * 128 + KD] *= scale
    # scatter channels into padded 1288 layout (see _build head slicing):
    # cols 0:768 q/k tiles (3 heads/tile at 0,32,64); 768:1288 v channels
    # hh*65+j with a ones channel (zero weight, bias 1) at j=64
    Wn = np.zeros((DIM, 1288), Wt.dtype)
    bn = np.zeros(1288, bvec.dtype)
    for hh in range(H):
        qdst = (hh // 3) * 128 + (hh % 3) * 32
        kdst = 384 + (hh // 3) * 128 + (hh % 3) * 32
        vdst = 768 + hh * 65
        qsrc, ksrc, vsrc = hh * 128, hh * 128 + 32, hh * 128 + 64
        Wn[:, qdst:qdst + 32] = Wt[:, qsrc:qsrc + 32]
        bn[qdst:qdst + 32] = bvec[qsrc:qsrc + 32]
        Wn[:, kdst:kdst + 32] = Wt[:, ksrc:ksrc + 32]
        bn[kdst:kdst + 32] = bvec[ksrc:ksrc + 32]
        Wn[:, vdst:vdst + 64] = Wt[:, vsrc:vsrc + 64]
        bn[vdst:vdst + 64] = bvec[vsrc:vsrc + 64]
        bn[vdst + 64] = 1.0
    return {
        "wqkv": np.ascontiguousarray(Wn).astype(BF16),
        "bqkv": np.ascontiguousarray(bn[:768].reshape(6, 128).T).astype(np.float32),
        "bv": np.ascontiguousarray(bn[768:].reshape(1, 520)).astype(np.float32),
        "wproj": np.ascontiguousarray(proj_w.T).astype(BF16),
        "wout": np.ascontiguousarray(out_w.T).astype(BF16),
        "tapw": np.ascontiguousarray(
            conv_w[:, 0].reshape(4, 128, 9).transpose(1, 0, 2).reshape(128, 36)
        ).astype(np.float32),
        "cbp": np.ascontiguousarray(conv_b.reshape(4, 128).T).astype(np.float32),
        "pbp": proj_b.reshape(1, IMG).astype(np.float32),
        "obp": out_b.reshape(1, DIM).astype(np.float32),
    }


def run(trace=False, tmpdir=None, **inputs):
    from concourse.bass_utils import run_bass_kernel_spmd

    if "nc" not in _cached:
        _cached["nc"] = _build()
    nc = _cached["nc"]

    w = _prep_weights(**{k: np.asarray(v) for k, v in inputs.items() if k != "x"})
    x = np.asarray(inputs["x"]).astype(BF16)
    in_maps = []
    for c in range(NCORES):
        m = dict(w)
        m["xs"] = np.ascontiguousarray(x[c * BPC:(c + 1) * BPC])
        in_maps.append(m)
    res = run_bass_kernel_spmd(
        nc, in_maps, core_ids=list(range(NCORES)), trace=trace, tmpdir=tmpdir
    )
    out = np.concatenate([np.asarray(r["out"]) for r in res.results], axis=0)
    return out.astype(np.float32), res.exec_time_ns


def kernel(**inputs):
    out, _ = run(trace=False, **inputs)
    return out


if __name__ == "__main__":
    print("building graph...")
    nc = _build()
    print("build OK:", len(nc.m.functions[0].allocations), "allocations")



# revision 8
# speedup vs baseline: 1.0262x; 1.0262x over previous
"""Trainium2 Bass kernel for nn_Attention_66546223284383.

Data-parallel over batch B=16 -> 2 batches per core x 8 cores.

v2: software-pipelined emission to keep the PE at K=8/8 (the baseline lost
~440us to HAM cold-clocking during attention):
  - per (head, m-tile): 3 score matmuls -> ONE fp32-PSUM exp over the full
    1044-col row (3-bank region, 2 regions ping-pong) -> ACT runs back-to-back.
  - attn@v runs as 3 detached accumulation chains per head (n-chunks
    512/512/20) streaming the SBUF et tiles; chains + PE transposes of head
    h-1 are emitted inside head h's window so the PE never idles long.
  - all non-score PSUM use (qkv, proj, out, transposes) goes through two
    1-bank rotating slots, so scores/exp never wait on other phases.
  - phase 1 of batch 1 and phase 3 of batch 0 are emitted as background
    groups drained inside the other batch's attention windows.
  - conv: fused scalar_tensor_tensor taps split across DVE and GpSimd,
    hardswish written straight into the padded tile, conv+proj bias folded
    into one precomputed tile.
"""
import sys
import numpy as np

sys.path.insert(0, "/opt/trn_rl_repo")

import ml_dtypes  # noqa: E402

BF16 = ml_dtypes.bfloat16

KD, H, D, DH, DIM, IMG, S, N, B = 32, 8, 64, 512, 512, 1024, 32, 1044, 16
EPS = 1e-5
NCORES = 8
BPC = B // NCORES  # batches per core
NT = [(i * 128, 128) for i in range(8)] + [(1024, 20)]   # 1044 partition tiles
NCH = [(0, 512), (512, 512), (1024, 20)]                 # 1044 free chunks

_cached = {}


def _build():
    from concourse import bacc, tile
    import concourse.bass as bass
    import concourse.mybir as mybir
    from concourse.masks import make_identity

    dt = mybir.dt
    alu = mybir.AluOpType
    act_exp = mybir.ActivationFunctionType.Exp
    act_id = mybir.ActivationFunctionType.Identity

    nc = bacc.Bacc(None, target_bir_lowering=False, debug=False)

    xs = nc.declare_dram_parameter("xs", [BPC, N, DIM], dt.bfloat16, isOutput=False)
    # wqkv cols: 0:768 = 6 padded qk tiles (3 q tiles then 3 k tiles, 3 heads
    # per tile at offsets 0/32/64); 768:1288 = v channels hh*65+j (j==64 is a
    # ones channel: zero weights, bias 1 -> softmax denominator column)
    wqkv = nc.declare_dram_parameter("wqkv", [DIM, 1288], dt.bfloat16, isOutput=False)
    bqkv = nc.declare_dram_parameter("bqkv", [128, 6], dt.float32, isOutput=False)
    bv = nc.declare_dram_parameter("bv", [1, 520], dt.float32, isOutput=False)
    wproj = nc.declare_dram_parameter("wproj", [N, IMG], dt.bfloat16, isOutput=False)
    wout = nc.declare_dram_parameter("wout", [DH, DIM], dt.bfloat16, isOutput=False)
    tapw = nc.declare_dram_parameter("tapw", [128, 36], dt.float32, isOutput=False)
    # combined conv_b (per channel/partition) + proj_b (per spatial col)
    cpb = nc.declare_dram_parameter("cpb", [128, 4, IMG], dt.bfloat16, isOutput=False)
    obp = nc.declare_dram_parameter("obp", [1, DIM], dt.float32, isOutput=False)
    out_ext = nc.declare_dram_parameter("out", [BPC, IMG, DIM], dt.float32, isOutput=True)

    o_nat = nc.dram_tensor("o_nat", [BPC, N, DH], dt.bfloat16)
    v_dram = nc.dram_tensor("v_dram", [BPC, H, N, D], dt.bfloat16)

    with tile.TileContext(nc) as tc:
        with (
            tc.tile_pool(name="w", bufs=1) as pw,
            tc.tile_pool(name="xT", bufs=5) as pxT,
            tc.tile_pool(name="qkvT", bufs=12) as pqk,
            tc.tile_pool(name="vnat", bufs=18) as pvn,
            tc.tile_pool(name="et", bufs=14) as pet,
            tc.tile_pool(name="oTs", bufs=2) as poT,
            tc.tile_pool(name="small", bufs=6) as psm,
            tc.tile_pool(name="conv", bufs=1) as pcv,
            tc.tile_pool(name="vc", bufs=8) as pvc,
            tc.tile_pool(name="o2t", bufs=10) as po2,
            tc.tile_pool(name="xo", bufs=5) as pxo,
            tc.tile_pool(name="osb", bufs=2) as pob,
            # PSUM budget (8 banks): 2 x 3-bank score regions + 2 x 1-bank
            tc.tile_pool(name="pssc", bufs=2, space=bass.MemorySpace.PSUM) as pssc,
            tc.tile_pool(name="psot", bufs=2, space=bass.MemorySpace.PSUM) as psot,
        ):
            # ---- constants / weights ----
            id_sb = pw.tile([128, 128], dt.bfloat16, tag="id")
            make_identity(nc, id_sb[:])
            wqkv_sb = pw.tile([128, 4, 1288], dt.bfloat16, tag="wqkv")
            nc.sync.dma_start(wqkv_sb[:], wqkv[:].rearrange("(k p) h -> p k h", p=128))
            bqkv_sb = pw.tile([128, 6], dt.float32, tag="bqkv")
            nc.sync.dma_start(bqkv_sb[:], bqkv[:])
            bv_sb = pw.tile([1, 520], dt.float32, tag="bv")
            nc.sync.dma_start(bv_sb[:], bv[:])
            bvbc = pw.tile([128, 520], dt.float32, tag="bvbc")
            nc.gpsimd.partition_broadcast(bvbc[:], bv_sb[:])
            wproj_sb = []
            for mt, (m0, msz) in enumerate(NT):
                t = pw.tile([msz, 1024], dt.bfloat16, tag=f"wproj{mt}")
                nc.sync.dma_start(t[:], wproj[m0:m0 + msz, :])
                wproj_sb.append(t)
            wout_sb = pw.tile([128, 4, DIM], dt.bfloat16, tag="wout")
            nc.sync.dma_start(wout_sb[:], wout[:].rearrange("(k p) c -> p k c", p=128))
            tapw_sb = pw.tile([128, 36], dt.float32, tag="tapw")
            nc.sync.dma_start(tapw_sb[:], tapw[:])
            cpb_sb = pw.tile([128, 4, IMG], dt.bfloat16, tag="cpb")
            nc.sync.dma_start(cpb_sb[:], cpb[:])
            ob_sb = pw.tile([1, DIM], dt.float32, tag="ob")
            nc.sync.dma_start(ob_sb[:], obp[:])
            obbc = pw.tile([128, DIM], dt.float32, tag="obbc")
            nc.gpsimd.partition_broadcast(obbc[:], ob_sb[:])
            # two persistent padded conv tiles; edges stay zero forever
            pads = []
            for k in range(2):
                t = pw.tile([128, 34, 34], dt.bfloat16, tag=f"pad{k}")
                nc.vector.memset(t[:], 0.0)
                pads.append(t)

            # ---------------- helpers ----------------
            xT_all, qkvT_all, vnat_all, et_all = {}, {}, {}, {}
            oT_all, vc_all, xo_all, o2t_all = {}, {}, {}, {}

            def emit_xT(b):
                xT = []
                for cb4 in range(4):
                    t = pxT.tile([128, N], dt.bfloat16, tag="xT", name=f"xT{b}_{cb4}")
                    c0 = cb4 * 128
                    nc.sync.dma_start_transpose(t[:, 0:1040], xs[b, 0:1040, c0:c0 + 128])
                    nc.sync.dma_start(
                        t[:, 1040:N], xs[b, 1040:N, c0:c0 + 128].rearrange("a b -> b a")
                    )
                    xT.append(t)
                xT_all[b] = xT

            def g_qk(b, t6, c):
                # one 512-col chunk of one padded qk tile; evac on ACT
                ci, cw = NCH[c]
                xT = xT_all[b]
                if t6 == 0 and c == 0:
                    qkvT_all[b] = [
                        pqk.tile([128, N], dt.bfloat16, tag="qkvT", name=f"qkvT{b}_{i}")
                        for i in range(6)
                    ]
                ps = psot.tile([128, 512], dt.float32, tag="ot", name=f"qk{b}_{t6}_{c}")
                for kc in range(4):
                    nc.tensor.matmul(
                        ps[:, 0:cw],
                        wqkv_sb[:, kc, t6 * 128:(t6 + 1) * 128],
                        xT[kc][:, ci:ci + cw],
                        start=(kc == 0), stop=(kc == 3),
                    )
                nc.scalar.activation(
                    qkvT_all[b][t6][:, ci:ci + cw], ps[:, 0:cw], act_id,
                    bias=bqkv_sb[:, t6:t6 + 1],
                )

            def g_vnat(b, nt, c):
                # one chunk (512 or 8 cols) of one vnat n-tile; evac on DVE
                n0, nsz = NT[nt]
                ci, cw = (0, 512) if c == 0 else (512, 8)
                xT = xT_all[b]
                if nt == 0 and c == 0:
                    vnat_all[b] = [
                        pvn.tile([NT[i][1], 520], dt.bfloat16, tag="vnat",
                                 name=f"vnat{b}_{i}")
                        for i in range(9)
                    ]
                ps = psot.tile([128, 512], dt.float32, tag="ot", name=f"vn{b}_{nt}_{c}")
                for kc in range(4):
                    nc.tensor.matmul(
                        ps[0:nsz, 0:cw],
                        xT[kc][:, n0:n0 + nsz],
                        wqkv_sb[:, kc, 768 + ci:768 + ci + cw],
                        start=(kc == 0), stop=(kc == 3),
                    )
                t = vnat_all[b][nt]
                nc.vector.tensor_tensor(
                    t[:, ci:ci + cw], ps[0:nsz, 0:cw], bvbc[0:nsz, ci:ci + cw],
                    op=alu.add,
                )
                if c == 1:
                    # stage all 8 heads' v for this n-tile in one DMA
                    nc.gpsimd.dma_start(
                        v_dram[b].rearrange("h n d -> n h d")[n0:n0 + nsz],
                        t[:].rearrange("p (h dd) -> p h dd", h=8)[:, :, 0:D],
                    )

            def emit_scores(b, hh, mt):
                # 3 score MMs + one exp for (head hh, m-tile mt)
                m0, msz = NT[mt]
                qT = qkvT_all[b][hh // 3]
                kT = qkvT_all[b][3 + hh // 3]
                qo = (hh % 3) * KD
                if mt == 0:
                    et_all[(b, hh)] = []
                reg = pssc.tile([128, 1536], dt.float32, tag="sc",
                                name=f"sc{b}_{hh}_{mt}")
                for (ci, cw) in NCH:
                    nc.tensor.matmul(
                        reg[0:msz, ci:ci + cw],
                        kT[qo:qo + KD, m0:m0 + msz],
                        qT[qo:qo + KD, ci:ci + cw],
                        start=True, stop=True,
                    )
                et = pet.tile([msz, N], dt.bfloat16, tag="et", name=f"et{b}_{hh}_{mt}")
                nc.scalar.activation(et[:, 0:N], reg[0:msz, 0:N], act_exp)
                et_all[(b, hh)].append(et)

            def g_chain(b, hh, c):
                # attn@v accumulation chain for n-chunk c of head hh (+evac)
                ci, cw = NCH[c]
                et = et_all[(b, hh)]
                vnat = vnat_all[b]
                if c == 0:
                    oT_all[(b, hh)] = poT.tile(
                        [D + 1, N], dt.bfloat16, tag="oTs", name=f"oT{b}_{hh}"
                    )
                ps = psot.tile([128, 512], dt.float32, tag="ot", name=f"ch{b}_{hh}_{c}")
                for tt in range(9):
                    nc.tensor.matmul(
                        ps[0:D + 1, 0:cw],
                        vnat[tt][:, hh * 65:hh * 65 + D + 1],
                        et[tt][:, ci:ci + cw],
                        start=(tt == 0), stop=(tt == 8),
                    )
                nc.vector.tensor_copy(oT_all[(b, hh)][:, ci:ci + cw], ps[0:D + 1, 0:cw])

            def g_trans(b, hh, nt):
                # transpose-back + normalize + stage o for one n-tile
                n0, nsz = NT[nt]
                oT = oT_all[(b, hh)]
                tp = psot.tile([128, D + 1], dt.bfloat16, tag="ot",
                               name=f"tp{b}_{hh}_{nt}")
                nc.tensor.transpose(
                    tp[0:nsz, 0:D + 1], oT[:, n0:n0 + nsz], id_sb[0:D + 1, 0:D + 1]
                )
                rcp = psm.tile([nsz, 1], dt.float32, tag="rcp")
                nc.vector.reciprocal(rcp[:], tp[0:nsz, D:D + 1])
                onrm = psm.tile([nsz, D], dt.bfloat16, tag="onrm")
                nc.vector.tensor_scalar(
                    onrm[:], tp[0:nsz, 0:D], rcp[:], None, op0=alu.mult
                )
                nc.gpsimd.dma_start(
                    o_nat[b, n0:n0 + nsz, hh * D:(hh + 1) * D], onrm[:]
                )

            def g_conv(b, ct, half):
                # half 0: load + hardswish into pad (gpsimd prep, DVE pad-mult)
                # half 1: 9 taps split DVE/gpsimd + combine + bias
                pad = pads[ct % 2]
                if half == 0:
                    cin = pcv.tile([128, 1024], dt.bfloat16, tag="cin", bufs=2)
                    for hl in range(2):
                        src = v_dram[b, 2 * ct + hl, 0:1024, :].rearrange(
                            "(c n2) d -> c (n2 d)", c=64
                        )
                        nc.gpsimd.dma_start(cin[hl * 64:(hl + 1) * 64, :], src)
                    u = pcv.tile([128, 1024], dt.bfloat16, tag="u")
                    nc.vector.tensor_scalar(
                        u[:], cin[:], 1.0 / 6.0, 0.5, op0=alu.mult, op1=alu.add
                    )
                    u2 = pcv.tile([128, 1024], dt.bfloat16, tag="u2")
                    nc.vector.tensor_scalar(
                        u2[:], u[:], 1.0, 0.0, op0=alu.min, op1=alu.max
                    )
                    nc.vector.tensor_tensor(
                        pad[:, 1:33, 1:33],
                        u2[:].rearrange("p (a b) -> p a b", a=32),
                        cin[:].rearrange("p (a b) -> p a b", a=32),
                        op=alu.mult,
                    )
                    return

                def chain(eng, taps, tag):
                    # alternating-dst multiply-accumulate over the tap list
                    tiles = [
                        pcv.tile([128, 32, 32], dt.bfloat16, tag=f"{tag}{k}",
                                 name=f"{tag}{k}")
                        for k in range(2)
                    ]
                    cur = None
                    for i, tap in enumerate(taps):
                        dy, dx = tap // 3, tap % 3
                        view = pad[:, dy:dy + 32, dx:dx + 32]
                        wsl = tapw_sb[:, ct * 9 + tap:ct * 9 + tap + 1]
                        dst = tiles[i % 2]
                        if cur is None:
                            eng.tensor_scalar(dst[:], view, wsl, None, op0=alu.mult)
                        else:
                            eng.scalar_tensor_tensor(
                                dst[:], view, wsl, cur[:], op0=alu.mult, op1=alu.add
                            )
                        cur = dst
                    return cur

                fd = chain(nc.vector, list(range(9)), "accd")
                vct = pvc.tile([128, 1024], dt.bfloat16, tag="vc", name=f"vc{b}_{ct}")
                nc.vector.tensor_tensor(
                    vct[:], fd[:].rearrange("p a b -> p (a b)"),
                    cpb_sb[:, ct, :], op=alu.add,
                )
                vc_all.setdefault(b, {})[ct] = vct

            def emit_o2t(b):
                O2v = o_nat[b].rearrange("n c -> (n c)").rearrange(
                    "(r m) -> r m", m=N
                )
                o2t = []
                for mt, (m0, msz) in enumerate(NT):
                    t = po2.tile([msz, DH], dt.bfloat16, tag="o2t", name=f"o2t{b}_{mt}")
                    if msz == 128:
                        nc.sync.dma_start_transpose(t[:], O2v[:, m0:m0 + msz])
                    else:
                        nc.sync.dma_start(
                            t[:], O2v[:, m0:m0 + msz].rearrange("a b -> b a")
                        )
                    o2t.append(t)
                o2t_all[b] = o2t

            def g_proj(b, rt, c2):
                o2t = o2t_all[b]
                jc = c2 * 512
                if rt == 0 and c2 == 0:
                    xo_all[b] = [
                        pxo.tile([128, 1024], dt.bfloat16, tag="xo",
                                 name=f"xo{b}_{i}")
                        for i in range(4)
                    ]
                ps = psot.tile([128, 512], dt.float32, tag="ot", name=f"pj{b}_{rt}_{c2}")
                for mt, (m0, msz) in enumerate(NT):
                    nc.tensor.matmul(
                        ps[:],
                        o2t[mt][0:msz, rt * 128:(rt + 1) * 128],
                        wproj_sb[mt][0:msz, jc:jc + 512],
                        start=(mt == 0), stop=(mt == 8),
                    )
                nc.vector.tensor_tensor(
                    xo_all[b][rt][:, jc:jc + 512], ps[:],
                    vc_all[b][rt][:, jc:jc + 512], op=alu.add,
                )

            def g_out(b, jt):
                xo = xo_all[b]
                ps = psot.tile([128, 512], dt.float32, tag="ot", name=f"ou{b}_{jt}")
                for rt in range(4):
                    nc.tensor.matmul(
                        ps[:],
                        xo[rt][:, jt * 128:(jt + 1) * 128],
                        wout_sb[:, rt, :],
                        start=(rt == 0), stop=(rt == 3),
                    )
                osb = pob.tile([128, DIM], dt.float32, tag="outsb")
                nc.vector.tensor_tensor(osb[:], ps[:], obbc[:], op=alu.add)
                nc.sync.dma_start(out_ext[b, jt * 128:(jt + 1) * 128, :], osb[:])

            # ---------------- emission schedule ----------------
            bg = []   # background PE groups (each = closure using one psot slot)

            def drain(k):
                for _ in range(min(k, len(bg))):
                    bg.pop(0)()

            # phase 1, batch 0 (direct)
            emit_xT(0)
            for t6 in range(6):
                for c in range(3):
                    g_qk(0, t6, c)
            for nt in range(9):
                for c in range(2):
                    g_vnat(0, nt, c)

            for b in range(BPC):
                # queue background work to overlay on this batch's attention:
                #   b=0 windows carry phase1(b=1); b=1 windows carry phase3(b=0)
                if b == 0:
                    emit_xT(1)
                    bg.extend(
                        (lambda t6=t6, c=c: g_qk(1, t6, c))
                        for t6 in range(6) for c in range(3)
                    )
                    bg.extend(
                        (lambda nt=nt, c=c: g_vnat(1, nt, c))
                        for nt in range(9) for c in range(2)
                    )
                else:
                    emit_o2t(0)
                    bg.extend(
                        (lambda rt=rt, c2=c2: g_proj(0, rt, c2))
                        for rt in range(4) for c2 in range(2)
                    )
                    bg.extend((lambda jt=jt: g_out(0, jt)) for jt in range(8))

                for hh in range(H):
                    # m-loop: scores + exp keep ACT saturated
                    for mt in range(9):
                        emit_scores(b, hh, mt)
                        # interleave previous head's consumption mid-window
                        if hh > 0:
                            ph = hh - 1
                            if mt in (1, 2, 3):
                                g_chain(b, ph, mt - 1)
                            elif mt == 4:
                                for nt in range(4):
                                    g_trans(b, ph, nt)
                                drain(2)
                            elif mt == 5:
                                for nt in range(4, 9):
                                    g_trans(b, ph, nt)
                            elif mt == 6:
                                g_conv(b, ph // 2, ph % 2)
                            elif mt == 7:
                                drain(3)
                    if hh == 0:
                        drain(1)
                # last head's consumption + remaining background
                for c in range(3):
                    g_chain(b, H - 1, c)
                for nt in range(9):
                    g_trans(b, H - 1, nt)
                g_conv(b, 3, 1)
                drain(len(bg))

            # phase 3, batch 1 (tail)
            emit_o2t(1)
            for rt in range(4):
                for c2 in range(2):
                    g_proj(1, rt, c2)
            for jt in range(8):
                g_out(1, jt)

    nc.compile()
    return nc


def _prep_weights(qkv_w, bn_gamma, bn_beta, bn_mean, bn_var,
                  conv_w, conv_b, proj_w, proj_b, out_w, out_b):
    s = bn_gamma / np.sqrt(bn_var + EPS)
    bias = bn_beta - bn_mean * s
    Wt = (qkv_w * s[:, None]).T.copy()
    bvec = bias.copy()
    scale = KD ** -0.5
    for hh in range(H):
        Wt[:, hh * 128:hh * 128 + KD] *= scale
        bvec[hh * 128:hh * 128 + KD] *= scale
    # scatter channels into padded 1288 layout (see _build head slicing):
    # cols 0:768 q/k tiles (3 heads/tile at 0,32,64); 768:1288 v channels
    # hh*65+j with a ones channel (zero weight, bias 1) at j=64
    Wn = np.zeros((DIM, 1288), Wt.dtype)
    bn = np.zeros(1288, bvec.dtype)
    for hh in range(H):
        qdst = (hh // 3) * 128 + (hh % 3) * 32
        kdst = 384 + (hh // 3) * 128 + (hh % 3) * 32
        vdst = 768 + hh * 65
        qsrc, ksrc, vsrc = hh * 128, hh * 128 + 32, hh * 128 + 64
        Wn[:, qdst:qdst + 32] = Wt[:, qsrc:qsrc + 32]
        bn[qdst:qdst + 32] = bvec[qsrc:qsrc + 32]
        Wn[:, kdst:kdst + 32] = Wt[:, ksrc:ksrc + 32]
        bn[kdst:kdst + 32] = bvec[ksrc:ksrc + 32]
        Wn[:, vdst:vdst + 64] = Wt[:, vsrc:vsrc + 64]
        bn[vdst:vdst + 64] = bvec[vsrc:vsrc + 64]
        bn[vdst + 64] = 1.0
    # combined conv bias (per channel) + proj bias (per spatial col)
    cpbn = (conv_b.reshape(4, 128, 1)
            + proj_b.reshape(1, 1, IMG)).transpose(1, 0, 2)
    return {
        "wqkv": np.ascontiguousarray(Wn).astype(BF16),
        "bqkv": np.ascontiguousarray(bn[:768].reshape(6, 128).T).astype(np.float32),
        "bv": np.ascontiguousarray(bn[768:].reshape(1, 520)).astype(np.float32),
        "wproj": np.ascontiguousarray(proj_w.T).astype(BF16),
        "wout": np.ascontiguousarray(out_w.T).astype(BF16),
        "tapw": np.ascontiguousarray(
            conv_w[:, 0].reshape(4, 128, 9).transpose(1, 0, 2).reshape(128, 36)
        ).astype(np.float32),
        "cpb": np.ascontiguousarray(cpbn).astype(BF16),
        "obp": out_b.reshape(1, DIM).astype(np.float32),
    }


def run(trace=False, tmpdir=None, **inputs):
    from concourse.bass_utils import run_bass_kernel_spmd

    if "nc" not in _cached:
        _cached["nc"] = _build()
    nc = _cached["nc"]

    w = _prep_weights(**{k: np.asarray(v) for k, v in inputs.items() if k != "x"})
    x = np.asarray(inputs["x"]).astype(BF16)
    in_maps = []
    for c in range(NCORES):
        m = dict(w)
        m["xs"] = np.ascontiguousarray(x[c * BPC:(c + 1) * BPC])
        in_maps.append(m)
    res = run_bass_kernel_spmd(
        nc, in_maps, core_ids=list(range(NCORES)), trace=trace, tmpdir=tmpdir
    )
    out = np.concatenate([np.asarray(r["out"]) for r in res.results], axis=0)
    return out.astype(np.float32), res.exec_time_ns


def kernel(**inputs):
    out, _ = run(trace=False, **inputs)
    return out


if __name__ == "__main__":
    print("building graph...")
    nc = _build()
    print("build OK:", len(nc.m.functions[0].allocations), "allocations")


# revision 22
# speedup vs baseline: 1.2380x; 1.2064x over previous
"""Trainium2 Bass kernel for nn_Attention_66546223284383.

Data-parallel over batch B=16 -> 2 batches per core x 8 cores.

v2: software-pipelined emission to keep the PE at K=8/8 (the baseline lost
~440us to HAM cold-clocking during attention):
  - per (head, m-tile): 3 score matmuls -> ONE fp32-PSUM exp over the full
    1044-col row (3-bank region, 2 regions ping-pong) -> ACT runs back-to-back.
  - attn@v runs as 3 detached accumulation chains per head (n-chunks
    512/512/20) streaming the SBUF et tiles; chains + PE transposes of head
    h-1 are emitted inside head h's window so the PE never idles long.
  - all non-score PSUM use (qkv, proj, out, transposes) goes through two
    1-bank rotating slots, so scores/exp never wait on other phases.
  - phase 1 of batch 1 and phase 3 of batch 0 are emitted as background
    groups drained inside the other batch's attention windows.
  - conv: fused scalar_tensor_tensor taps split across DVE and GpSimd,
    hardswish written straight into the padded tile, conv+proj bias folded
    into one precomputed tile.
"""
import sys
import numpy as np

sys.path.insert(0, "/opt/trn_rl_repo")

import ml_dtypes  # noqa: E402

BF16 = ml_dtypes.bfloat16

KD, H, D, DH, DIM, IMG, S, N, B = 32, 8, 64, 512, 512, 1024, 32, 1044, 16
EPS = 1e-5
NCORES = 8
BPC = B // NCORES  # batches per core
NT = [(i * 128, 128) for i in range(8)] + [(1024, 20)]   # 1044 partition tiles
NCH = [(0, 512), (512, 512), (1024, 20)]                 # 1044 free chunks

_cached = {}


def _build():
    from concourse import bacc, tile
    import concourse.bass as bass
    import concourse.mybir as mybir
    from concourse.masks import make_identity

    dt = mybir.dt
    alu = mybir.AluOpType
    act_exp = mybir.ActivationFunctionType.Exp
    act_id = mybir.ActivationFunctionType.Identity

    nc = bacc.Bacc(None, target_bir_lowering=False, debug=False)

    xs = nc.declare_dram_parameter("xs", [BPC, N, DIM], dt.bfloat16, isOutput=False)
    # wqkv cols: 0:768 = 6 padded qk tiles (3 q tiles then 3 k tiles, 3 heads
    # per tile at offsets 0/32/64); 768:1288 = v channels hh*65+j (j==64 is a
    # ones channel: zero weights, bias 1 -> softmax denominator column)
    wqkv = nc.declare_dram_parameter("wqkv", [DIM, 1288], dt.bfloat16, isOutput=False)
    bqkv = nc.declare_dram_parameter("bqkv", [128, 6], dt.float32, isOutput=False)
    bv = nc.declare_dram_parameter("bv", [1, 520], dt.float32, isOutput=False)
    wproj = nc.declare_dram_parameter("wproj", [N, IMG], dt.bfloat16, isOutput=False)
    wout = nc.declare_dram_parameter("wout", [DH, DIM], dt.bfloat16, isOutput=False)
    tapw = nc.declare_dram_parameter("tapw", [128, 36], dt.float32, isOutput=False)
    # combined conv_b (per channel/partition) + proj_b (per spatial col)
    cpb = nc.declare_dram_parameter("cpb", [128, 4, IMG], dt.bfloat16, isOutput=False)
    obp = nc.declare_dram_parameter("obp", [1, DIM], dt.float32, isOutput=False)
    out_ext = nc.declare_dram_parameter("out", [BPC, IMG, DIM], dt.float32, isOutput=True)

    o_nat = nc.dram_tensor("o_nat", [BPC, N, DH], dt.bfloat16)
    v_dram = nc.dram_tensor("v_dram", [BPC, H, N, D], dt.bfloat16)

    with tile.TileContext(nc) as tc:
        with (
            tc.tile_pool(name="w", bufs=1) as pw,
            tc.tile_pool(name="xT", bufs=5) as pxT,
            tc.tile_pool(name="qkvT", bufs=12) as pqk,
            tc.tile_pool(name="vnat", bufs=18) as pvn,
            tc.tile_pool(name="et", bufs=20) as pet,
            tc.tile_pool(name="oTs", bufs=2) as poT,
            tc.tile_pool(name="small", bufs=4) as psm,
            tc.tile_pool(name="conv", bufs=1) as pcv,
            tc.tile_pool(name="vc", bufs=8) as pvc,
            tc.tile_pool(name="o2t", bufs=9) as po2,
            tc.tile_pool(name="xo", bufs=4) as pxo,
            tc.tile_pool(name="osb", bufs=2) as pob,
            # PSUM budget (8 banks): 2 x 3-bank score regions + 2 x 1-bank
            tc.tile_pool(name="pssc", bufs=2, space=bass.MemorySpace.PSUM) as pssc,
            tc.tile_pool(name="psot", bufs=2, space=bass.MemorySpace.PSUM) as psot,
        ):
            # ---- constants / weights ----
            id_sb = pw.tile([128, 128], dt.bfloat16, tag="id")
            make_identity(nc, id_sb[:])
            wqkv_sb = pw.tile([128, 4, 1288], dt.bfloat16, tag="wqkv")
            nc.sync.dma_start(wqkv_sb[:], wqkv[:].rearrange("(k p) h -> p k h", p=128))
            bqkv_sb = pw.tile([128, 6], dt.float32, tag="bqkv")
            nc.sync.dma_start(bqkv_sb[:], bqkv[:])
            bv_sb = pw.tile([1, 520], dt.float32, tag="bv")
            nc.sync.dma_start(bv_sb[:], bv[:])
            bvbc = pw.tile([128, 520], dt.float32, tag="bvbc")
            nc.gpsimd.partition_broadcast(bvbc[:], bv_sb[:])
            wproj_sb = []
            for mt, (m0, msz) in enumerate(NT):
                t = pw.tile([msz, 1024], dt.bfloat16, tag=f"wproj{mt}")
                nc.sync.dma_start(t[:], wproj[m0:m0 + msz, :])
                wproj_sb.append(t)
            wout_sb = pw.tile([128, 4, DIM], dt.bfloat16, tag="wout")
            nc.sync.dma_start(wout_sb[:], wout[:].rearrange("(k p) c -> p k c", p=128))
            tapw_sb = pw.tile([128, 36], dt.float32, tag="tapw")
            nc.sync.dma_start(tapw_sb[:], tapw[:])
            cpb_sb = pw.tile([128, 4, IMG], dt.bfloat16, tag="cpb")
            nc.sync.dma_start(cpb_sb[:], cpb[:])
            ob_sb = pw.tile([1, DIM], dt.float32, tag="ob")
            nc.sync.dma_start(ob_sb[:], obp[:])
            obbc = pw.tile([128, DIM], dt.float32, tag="obbc")
            nc.gpsimd.partition_broadcast(obbc[:], ob_sb[:])
            # two persistent padded conv tiles; edges stay zero forever
            pads = []
            for k in range(2):
                t = pw.tile([128, 34, 34], dt.bfloat16, tag=f"pad{k}")
                nc.vector.memset(t[:], 0.0)
                pads.append(t)

            # ---------------- helpers ----------------
            xT_all, qkvT_all, vnat_all, et_all = {}, {}, {}, {}
            oT_all, vc_all, xo_all, o2t_all = {}, {}, {}, {}

            def emit_xT(b):
                xT = []
                for cb4 in range(4):
                    t = pxT.tile([128, N], dt.bfloat16, tag="xT", name=f"xT{b}_{cb4}")
                    c0 = cb4 * 128
                    nc.sync.dma_start_transpose(t[:, 0:1040], xs[b, 0:1040, c0:c0 + 128])
                    nc.sync.dma_start(
                        t[:, 1040:N], xs[b, 1040:N, c0:c0 + 128].rearrange("a b -> b a")
                    )
                    xT.append(t)
                xT_all[b] = xT

            def g_qk(b, t6, c):
                # one 512-col chunk of one padded qk tile; evac on ACT
                ci, cw = NCH[c]
                xT = xT_all[b]
                if t6 == 0 and c == 0:
                    qkvT_all[b] = [
                        pqk.tile([128, N], dt.bfloat16, tag="qkvT", name=f"qkvT{b}_{i}")
                        for i in range(6)
                    ]
                ps = psot.tile([128, 512], dt.float32, tag="ot", name=f"qk{b}_{t6}_{c}")
                for kc in range(4):
                    nc.tensor.matmul(
                        ps[:, 0:cw],
                        wqkv_sb[:, kc, t6 * 128:(t6 + 1) * 128],
                        xT[kc][:, ci:ci + cw],
                        start=(kc == 0), stop=(kc == 3),
                    )
                nc.scalar.activation(
                    qkvT_all[b][t6][:, ci:ci + cw], ps[:, 0:cw], act_id,
                    bias=bqkv_sb[:, t6:t6 + 1],
                )

            def g_vnat(b, nt, c):
                # one chunk (512 or 8 cols) of one vnat n-tile; evac on DVE
                n0, nsz = NT[nt]
                ci, cw = (0, 512) if c == 0 else (512, 8)
                xT = xT_all[b]
                if nt == 0 and c == 0:
                    vnat_all[b] = [
                        pvn.tile([NT[i][1], 520], dt.bfloat16, tag="vnat",
                                 name=f"vnat{b}_{i}")
                        for i in range(9)
                    ]
                ps = psot.tile([128, 512], dt.float32, tag="ot", name=f"vn{b}_{nt}_{c}")
                for kc in range(4):
                    nc.tensor.matmul(
                        ps[0:nsz, 0:cw],
                        xT[kc][:, n0:n0 + nsz],
                        wqkv_sb[:, kc, 768 + ci:768 + ci + cw],
                        start=(kc == 0), stop=(kc == 3),
                    )
                t = vnat_all[b][nt]
                nc.vector.tensor_tensor(
                    t[:, ci:ci + cw], ps[0:nsz, 0:cw], bvbc[0:nsz, ci:ci + cw],
                    op=alu.add,
                )
                if c == 1:
                    # stage all 8 heads' v for this n-tile in one DMA
                    nc.gpsimd.dma_start(
                        v_dram[b].rearrange("h n d -> n h d")[n0:n0 + nsz],
                        t[:].rearrange("p (h dd) -> p h dd", h=8)[:, :, 0:D],
                    )

            def emit_scores(b, hh, mt):
                # 3 score MMs + one exp for (head hh, m-tile mt)
                m0, msz = NT[mt]
                qT = qkvT_all[b][hh // 3]
                kT = qkvT_all[b][3 + hh // 3]
                qo = (hh % 3) * KD
                if mt == 0:
                    et_all[(b, hh)] = []
                reg = pssc.tile([128, 1536], dt.float32, tag="sc",
                                name=f"sc{b}_{hh}_{mt}")
                for (ci, cw) in NCH:
                    nc.tensor.matmul(
                        reg[0:msz, ci:ci + cw],
                        kT[qo:qo + KD, m0:m0 + msz],
                        qT[qo:qo + KD, ci:ci + cw],
                        start=True, stop=True,
                    )
                et = pet.tile([msz, N], dt.bfloat16, tag="et", name=f"et{b}_{hh}_{mt}")
                nc.scalar.activation(et[:, 0:N], reg[0:msz, 0:N], act_exp)
                et_all[(b, hh)].append(et)

            def g_chain(b, hh, c):
                # attn@v accumulation chain for n-chunk c of head hh (+evac)
                ci, cw = NCH[c]
                et = et_all[(b, hh)]
                vnat = vnat_all[b]
                if c == 0:
                    oT_all[(b, hh)] = poT.tile(
                        [D + 1, N], dt.bfloat16, tag="oTs", name=f"oT{b}_{hh}"
                    )
                ps = psot.tile([128, 512], dt.float32, tag="ot", name=f"ch{b}_{hh}_{c}")
                for tt in range(9):
                    nc.tensor.matmul(
                        ps[0:D + 1, 0:cw],
                        vnat[tt][:, hh * 65:hh * 65 + D + 1],
                        et[tt][:, ci:ci + cw],
                        start=(tt == 0), stop=(tt == 8),
                    )
                nc.vector.tensor_copy(oT_all[(b, hh)][:, ci:ci + cw], ps[0:D + 1, 0:cw])

            def g_post(b, hh):
                oT = oT_all[(b, hh)]
                for nt, (n0, nsz) in enumerate(NT):
                    tp = psot.tile([128, D + 1], dt.bfloat16, tag="ot",
                                   name=f"tp{b}_{hh}_{nt}")
                    nc.tensor.transpose(
                        tp[0:nsz, 0:D + 1], oT[:, n0:n0 + nsz],
                        id_sb[0:D + 1, 0:D + 1],
                    )
                    rcp = psm.tile([nsz, 1], dt.float32, tag="rcp")
                    nc.vector.reciprocal(rcp[:], tp[0:nsz, D:D + 1])
                    onrm = psm.tile([nsz, D], dt.bfloat16, tag="onrm")
                    nc.vector.tensor_scalar(
                        onrm[:], tp[0:nsz, 0:D], rcp[:], None, op0=alu.mult
                    )
                    nc.gpsimd.dma_start(
                        o_nat[b, n0:n0 + nsz, hh * D:(hh + 1) * D], onrm[:]
                    )

            def g_conv(b, ct, half):
                # half 0: load + hardswish into pad (gpsimd prep, DVE pad-mult)
                # half 1: 9 taps split DVE/gpsimd + combine + bias
                pad = pads[ct % 2]
                if half == 0:
                    cin = pcv.tile([128, 1024], dt.bfloat16, tag="cin", bufs=2)
                    for hl in range(2):
                        nc.gpsimd.dma_start(
                            cin[hl * 64:(hl + 1) * 64, :],
                            v_dram[b, 2 * ct + hl, 0:1024, :].rearrange(
                                "(c n2) d -> c (n2 d)", c=64
                            ),
                        )
                    u = pcv.tile([128, 1024], dt.bfloat16, tag="u")
                    nc.vector.tensor_scalar(
                        u[:], cin[:], 1.0 / 6.0, 0.5, op0=alu.mult, op1=alu.add
                    )
                    u2 = pcv.tile([128, 1024], dt.bfloat16, tag="u2")
                    nc.vector.tensor_scalar(
                        u2[:], u[:], 1.0, 0.0, op0=alu.min, op1=alu.max
                    )
                    nc.vector.tensor_tensor(
                        pad[:, 1:33, 1:33],
                        u2[:].rearrange("p (a b) -> p a b", a=32),
                        cin[:].rearrange("p (a b) -> p a b", a=32),
                        op=alu.mult,
                    )
                    return

                acc = pcv.tile([128, 1024], dt.bfloat16, tag="acc", name="acc")
                tmp = pcv.tile([128, 1024], dt.bfloat16, tag="tmp", name="tmp")
                for tap in range(9):
                    dy, dx = tap // 3, tap % 3
                    view = pad[:, dy:dy + 32, dx:dx + 32]
                    wsl = tapw_sb[:, ct * 9 + tap:ct * 9 + tap + 1]
                    dst = acc if tap == 0 else tmp
                    nc.vector.tensor_scalar(
                        dst[:].rearrange("p (a b) -> p a b", a=32),
                        view, wsl, None, op0=alu.mult,
                    )
                    if tap:
                        nc.vector.tensor_tensor(acc[:], acc[:], tmp[:], op=alu.add)
                vct = pvc.tile([128, 1024], dt.bfloat16, tag="vc", name=f"vc{b}_{ct}")
                nc.vector.tensor_tensor(vct[:], acc[:], cpb_sb[:, ct, :], op=alu.add)
                vc_all.setdefault(b, {})[ct] = vct

            def emit_o2t(b):
                O2v = o_nat[b].rearrange("n c -> (n c)").rearrange(
                    "(r m) -> r m", m=N
                )
                o2t = []
                for mt, (m0, msz) in enumerate(NT):
                    t = po2.tile([msz, DH], dt.bfloat16, tag="o2t", name=f"o2t{b}_{mt}")
                    if msz == 128:
                        nc.sync.dma_start_transpose(t[:], O2v[:, m0:m0 + msz])
                    else:
                        nc.sync.dma_start(
                            t[:], O2v[:, m0:m0 + msz].rearrange("a b -> b a")
                        )
                    o2t.append(t)
                o2t_all[b] = o2t

            def g_proj(b, rt, c2):
                o2t = o2t_all[b]
                jc = c2 * 512
                if rt == 0 and c2 == 0:
                    xo_all[b] = [
                        pxo.tile([128, 1024], dt.bfloat16, tag="xo",
                                 name=f"xo{b}_{i}")
                        for i in range(4)
                    ]
                ps = psot.tile([128, 512], dt.float32, tag="ot", name=f"pj{b}_{rt}_{c2}")
                for mt, (m0, msz) in enumerate(NT):
                    nc.tensor.matmul(
                        ps[:],
                        o2t[mt][0:msz, rt * 128:(rt + 1) * 128],
                        wproj_sb[mt][0:msz, jc:jc + 512],
                        start=(mt == 0), stop=(mt == 8),
                    )
                nc.vector.tensor_tensor(
                    xo_all[b][rt][:, jc:jc + 512], ps[:],
                    vc_all[b][rt][:, jc:jc + 512], op=alu.add,
                )

            def g_out(b, jt):
                xo = xo_all[b]
                ps = psot.tile([128, 512], dt.float32, tag="ot", name=f"ou{b}_{jt}")
                for rt in range(4):
                    nc.tensor.matmul(
                        ps[:],
                        xo[rt][:, jt * 128:(jt + 1) * 128],
                        wout_sb[:, rt, :],
                        start=(rt == 0), stop=(rt == 3),
                    )
                osb = pob.tile([128, DIM], dt.float32, tag="outsb")
                nc.vector.tensor_tensor(osb[:], ps[:], obbc[:], op=alu.add)
                nc.sync.dma_start(out_ext[b, jt * 128:(jt + 1) * 128, :], osb[:])

            # ---------------- emission schedule ----------------
            bg = []   # background PE groups (each = closure using one psot slot)

            def drain(k):
                for _ in range(min(k, len(bg))):
                    bg.pop(0)()

            # phase 1, batch 0 (direct)
            emit_xT(0)
            for t6 in range(6):
                for c in range(3):
                    g_qk(0, t6, c)
            for nt in range(9):
                for c in range(2):
                    g_vnat(0, nt, c)

            for b in range(BPC):
                # queue background work to overlay on this batch's attention:
                #   b=0 windows carry phase1(b=1); b=1 windows carry phase3(b=0)
                if b == 0:
                    emit_xT(1)
                    bg.extend(
                        (lambda t6=t6, c=c: g_qk(1, t6, c))
                        for t6 in range(6) for c in range(3)
                    )
                    bg.extend(
                        (lambda nt=nt, c=c: g_vnat(1, nt, c))
                        for nt in range(9) for c in range(2)
                    )
                else:
                    emit_o2t(0)
                    bg.extend(
                        (lambda rt=rt, c2=c2: g_proj(0, rt, c2))
                        for rt in range(4) for c2 in range(2)
                    )
                    bg.extend((lambda jt=jt: g_out(0, jt)) for jt in range(8))

                for hh in range(H):
                    for mt in range(9):
                        emit_scores(b, hh, mt)
                        if hh > 0:
                            ph = hh - 1
                            if mt in (1, 2, 3):
                                g_chain(b, ph, mt - 1)
                            elif mt == 5:
                                g_post(b, ph)
                            elif mt == 6:
                                g_conv(b, ph // 2, ph % 2)
                            elif mt == 7:
                                drain(3)
                    if hh == 0:
                        drain(2)
                # last head's consumption + remaining background
                for c in range(3):
                    g_chain(b, H - 1, c)
                g_post(b, H - 1)
                g_conv(b, (H - 1) // 2, (H - 1) % 2)
                drain(len(bg))

            # phase 3, batch 1 (tail)
            emit_o2t(1)
            for rt in range(4):
                for c2 in range(2):
                    g_proj(1, rt, c2)
            for jt in range(8):
                g_out(1, jt)

    nc.compile()
    return nc


def _prep_weights(qkv_w, bn_gamma, bn_beta, bn_mean, bn_var,
                  conv_w, conv_b, proj_w, proj_b, out_w, out_b):
    s = bn_gamma / np.sqrt(bn_var + EPS)
    bias = bn_beta - bn_mean * s
    Wt = (qkv_w * s[:, None]).T.copy()
    bvec = bias.copy()
    scale = KD ** -0.5
    for hh in range(H):
        Wt[:, hh * 128:hh * 128 + KD] *= scale
        bvec[hh * 128:hh * 128 + KD] *= scale
    # scatter channels into padded 1288 layout (see _build head slicing):
    # cols 0:768 q/k tiles (3 heads/tile at 0,32,64); 768:1288 v channels
    # hh*65+j with a ones channel (zero weight, bias 1) at j=64
    Wn = np.zeros((DIM, 1288), Wt.dtype)
    bn = np.zeros(1288, bvec.dtype)
    for hh in range(H):
        qdst = (hh // 3) * 128 + (hh % 3) * 32
        kdst = 384 + (hh // 3) * 128 + (hh % 3) * 32
        vdst = 768 + hh * 65
        qsrc, ksrc, vsrc = hh * 128, hh * 128 + 32, hh * 128 + 64
        Wn[:, qdst:qdst + 32] = Wt[:, qsrc:qsrc + 32]
        bn[qdst:qdst + 32] = bvec[qsrc:qsrc + 32]
        Wn[:, kdst:kdst + 32] = Wt[:, ksrc:ksrc + 32]
        bn[kdst:kdst + 32] = bvec[ksrc:ksrc + 32]
        Wn[:, vdst:vdst + 64] = Wt[:, vsrc:vsrc + 64]
        bn[vdst:vdst + 64] = bvec[vsrc:vsrc + 64]
        bn[vdst + 64] = 1.0
    # combined conv bias (per channel) + proj bias (per spatial col)
    cpbn = (conv_b.reshape(4, 128, 1)
            + proj_b.reshape(1, 1, IMG)).transpose(1, 0, 2)
    return {
        "wqkv": np.ascontiguousarray(Wn).astype(BF16),
        "bqkv": np.ascontiguousarray(bn[:768].reshape(6, 128).T).astype(np.float32),
        "bv": np.ascontiguousarray(bn[768:].reshape(1, 520)).astype(np.float32),
        "wproj": np.ascontiguousarray(proj_w.T).astype(BF16),
        "wout": np.ascontiguousarray(out_w.T).astype(BF16),
        "tapw": np.ascontiguousarray(
            conv_w[:, 0].reshape(4, 128, 9).transpose(1, 0, 2).reshape(128, 36)
        ).astype(np.float32),
        "cpb": np.ascontiguousarray(cpbn).astype(BF16),
        "obp": out_b.reshape(1, DIM).astype(np.float32),
    }


def run(trace=False, tmpdir=None, **inputs):
    from concourse.bass_utils import run_bass_kernel_spmd

    if "nc" not in _cached:
        _cached["nc"] = _build()
    nc = _cached["nc"]

    w = _prep_weights(**{k: np.asarray(v) for k, v in inputs.items() if k != "x"})
    x = np.asarray(inputs["x"]).astype(BF16)
    in_maps = []
    for c in range(NCORES):
        m = dict(w)
        m["xs"] = np.ascontiguousarray(x[c * BPC:(c + 1) * BPC])
        in_maps.append(m)
    res = run_bass_kernel_spmd(
        nc, in_maps, core_ids=list(range(NCORES)), trace=trace, tmpdir=tmpdir
    )
    out = np.concatenate([np.asarray(r["out"]) for r in res.results], axis=0)
    return out.astype(np.float32), res.exec_time_ns


def kernel(**inputs):
    out, _ = run(trace=False, **inputs)
    return out


if __name__ == "__main__":
    print("building graph...")
    nc = _build()
    print("build OK:", len(nc.m.functions[0].allocations), "allocations")
